# revision 3
# baseline (speedup 1.0000x reference)
"""Trainium2 kernel for nn_DilatedReparamBlock_21139829031531.

Math: the block is 7 depthwise-conv+BN branches summed. All branches merge
("reparameterize") exactly into ONE depthwise 13x13 conv + per-channel bias:
    K_c = sum_i scale_i[c] * dilate(w_i[c])   (placed centered in 13x13)
    bias_c = sum_i (beta_i - mean_i * scale_i)[c]
The host folds weights/BN (tiny), the device runs the single conv.

Device scheme ("row-pair Toeplitz"): channels sharded 32/core across 8 cores.
Per channel, SBUF holds x with partitions = (h%2)*56 + w (a row-PAIR of the
image across 112 partitions). The 13x13 depthwise conv becomes 7 PE matmuls
accumulating in PSUM:
    out[(a_out,w_out), (b,t)] += sum_{a_in,w_in}
        lhsT_j[(a_in,w_in), (a_out,w_out)] * x[b, 2(t+j2)+a_in, w_in]
where lhsT_j[(ai,wi),(ao,wo)] = K[2*j2+ai-ao+6, wi-wo+6] (0 outside the
13-tap band; W/H zero-padding falls out of band clipping + t clipping).
fp16 operands, fp32 PSUM accumulate, fp16 output + host fp32 cast.

Scheduling is hand-rolled raw Bass (no TileContext): every buffer is
statically allocated (no reuse except PSUM's 8-bank rotation), and six
monotonic counting semaphores order the pipeline:
    SX / SW: per-channel x / weight DMA completions (in-order per queue)
    SB:      bias DMA completion
    SM:      tensor marks each (channel, batch-half) PSUM group done
    SC:      vector marks each PSUM->SBUF bias-add copy done
    SO:      output DMA completions (final quiesce waits SO == 64)
This keeps the framework's ~7us entry barrier chain and ~10us teardown
semaphore walk out of the NEFF: the head is just iram load + first DMA,
the tail a single barrier + 6 sem clears + barrier.
"""

import os
import sys

import numpy as np

for _p in ("/opt/trn_rl_repo", "/root/.axon_site/_ro/trn_rl_repo"):
    if os.path.isdir(_p) and _p not in sys.path:
        sys.path.append(_p)

import contextlib

import concourse.bass as bass
import concourse.bacc as bacc
import concourse.mybir as mybir
from concourse.bass_utils import run_bass_kernel_spmd

# ---------------------------------------------------------------- constants
C = 256
B = 32
H = W = 56
M = H // 2            # row pairs
NCORES = 8
CL = C // NCORES      # channels per core
KS = [5, 7, 7, 3, 3, 3]
DIL = [1, 1, 2, 3, 4, 5]
EPS = 1e-5
J2S = [0, -3, -2, -1, 1, 2, 3]   # j2 = (row-pair offset); row offset = 2*j2
F16 = mybir.dt.float16
F32 = mybir.dt.float32
NWARM = 6             # PE clock-ramp warmup matmuls (garbage data)


# ------------------------------------------------------------- host math
def _merge_weights(lk_w, ws, bn_gamma, bn_beta, bn_mean, bn_var):
    """Fold all branches + BN into one [C,13,13] kernel and [C] bias."""
    g = bn_gamma.astype(np.float64)
    be = bn_beta.astype(np.float64)
    mu = bn_mean.astype(np.float64)
    va = bn_var.astype(np.float64)
    scale = g / np.sqrt(va + EPS)          # [7, C]
    shift = be - mu * scale                # [7, C]
    K = np.zeros((C, 13, 13), np.float64)
    K += scale[0][:, None, None] * lk_w[:, 0].astype(np.float64)
    for i, (k, r) in enumerate(zip(KS, DIL)):
        w = ws[i][:, 0].astype(np.float64)
        span = r * (k - 1) + 1
        off = (13 - span) // 2
        ii = off + r * np.arange(k)
        K[:, ii[:, None], ii[None, :]] += scale[i + 1][:, None, None] * w
    bias = shift.sum(axis=0)               # [C]
    return K, bias


def _build_toeplitz(K):
    """lhsT for all channels: [C, 7, 112, 112] fp16.

    lhsT[c, j, p=(ai*56+wi), f=(ao*56+wo)] = K[c, 2*j2+ai-ao+6, wi-wo+6]
    """
    p = np.arange(112)
    f = np.arange(112)
    ai, wi = p // 56, p % 56
    ao, wo = f // 56, f % 56
    dx = wi[:, None] - wo[None, :]                       # [112,112]
    lhs = np.zeros((C, 7, 112, 112), np.float32)
    for j, j2 in enumerate(J2S):
        dy = 2 * j2 + ai[:, None] - ao[None, :]          # [112,112]
        valid = (np.abs(dy) <= 6) & (np.abs(dx) <= 6)
        dyc = np.clip(dy + 6, 0, 12)
        dxc = np.clip(dx + 6, 0, 12)
        vals = K[:, dyc, dxc] * valid[None]              # [C,112,112]
        lhs[:, j] = vals
    return lhs.astype(np.float16)


def _stage_inputs(x, K, bias):
    """Per-core xs [112, CL, B, M] fp16, wt [112, CL, 7, 112] fp16, and a
    bias plane [112, CL] fp32 (bias replicated across partitions so the
    PSUM->SBUF copy can add it as a per-partition tensor_scalar operand).
    Partition counts stay multiples of 16 — the DMA engine striping
    serializes onto one SDMA engine otherwise."""
    lhs = _build_toeplitz(K)                             # [C, 7, 112, 112]
    xr = (
        x.reshape(B, NCORES, CL, M, 2, W)
        .transpose(4, 5, 1, 2, 0, 3)                     # [a, w, core, cl, b, m]
        .reshape(112, NCORES, CL, B, M)
        .astype(np.float16)
    )
    xs_l, wt_l, bi_l = [], [], []
    for core in range(NCORES):
        xs_l.append(np.ascontiguousarray(xr[:, core]))
        wc = lhs[core * CL:(core + 1) * CL].transpose(2, 0, 1, 3)  # [112,CL,7,112]
        wt_l.append(np.ascontiguousarray(wc))
        bc = np.broadcast_to(
            bias[core * CL:(core + 1) * CL].astype(np.float32)[None, :], (112, CL)
        )
        bi_l.append(np.ascontiguousarray(bc))
    return xs_l, wt_l, bi_l


def _unstage(outs):
    """outs: list of 8 arrays [112, CL, B, M] fp16 -> [B, C, H, W] fp32."""
    O = np.stack(outs).astype(np.float32)                # [8, 112, CL, B, M]
    return np.ascontiguousarray(
        O.reshape(NCORES, 2, W, CL, B, M)
        .transpose(4, 0, 3, 5, 1, 2)                     # [B, core, cl, m, a, w]
        .reshape(B, C, H, W)
    )


# --------------------------------------------------------- device program
def _build_program():
    nc = bacc.Bacc()
    xs = nc.declare_dram_parameter("xs", [112, CL, B, M], F16, isOutput=False)
    wt = nc.declare_dram_parameter("wt", [112, CL, 7, 112], F16, isOutput=False)
    bi = nc.declare_dram_parameter("bi", [112, CL], F32, isOutput=False)
    out = nc.declare_dram_parameter("out", [112, CL, B, M], F16, isOutput=True)

    sx = nc.alloc_semaphore("sx")   # x-channel DMA completions
    sw = nc.alloc_semaphore("sw")   # weight-channel DMA completions
    sb_ = nc.alloc_semaphore("sb")  # bias DMA completion
    sm = nc.alloc_semaphore("sm")   # (channel, half) matmul groups retired
    sc = nc.alloc_semaphore("sc")   # (channel, half) copies retired
    so = nc.alloc_semaphore("so")   # output DMA completions

    ctx = contextlib.ExitStack()
    with ctx:
        xt = [ctx.enter_context(nc.sbuf_tensor(f"xt{c}", [112, B, M], F16))
              for c in range(CL)]
        wtt = [ctx.enter_context(nc.sbuf_tensor(f"wt{c}", [112, 7, 112], F16))
               for c in range(CL)]
        sbo = [ctx.enter_context(nc.sbuf_tensor(f"ot{c}", [112, B, M], F16))
               for c in range(CL)]
        bias_t = ctx.enter_context(nc.sbuf_tensor("bias", [112, CL], F32))
        warm = ctx.enter_context(nc.sbuf_tensor("warm", [112, 512], F16))
        ps = [ctx.enter_context(nc.psum_tensor(f"ps{k}", [112, 16, M], F32))
              for k in range(8)]

        # --- DMA issue queues.  sync: x in + results out; gpsimd: weights+bias.
        for c in range(CL):
            nc.sync.dma_start(xt[c][:], xs[:, c]).then_inc(sx, 16)
        nc.gpsimd.dma_start(wtt[0][:], wt[:, 0]).then_inc(sw, 16)
        nc.gpsimd.dma_start(bias_t[:], bi[:]).then_inc(sb_, 16)
        for c in range(1, CL):
            nc.gpsimd.dma_start(wtt[c][:], wt[:, c]).then_inc(sw, 16)

        # --- Tensor engine: clock-ramp warmup, then the real matmul stream.
        for _ in range(NWARM):
            nc.tensor.matmul(ps[0][:], warm[:, 0:112], warm[:, 0:448],
                             start=True, stop=True)
        for c in range(CL):
            nc.tensor.wait_ge(sx, 16 * (c + 1))
            nc.tensor.wait_ge(sw, 16 * (c + 1))
            if c >= 4:
                # PSUM bank pair (2c)%8,(2c+1)%8 last read by channel c-4's
                # copies; SC counts 2 per channel.
                nc.tensor.wait_ge(sc, 2 * c - 6)
            for ph, b0 in ((0, 0), (1, 16)):
                pst = ps[(2 * c + ph) % 8]
                for j, j2 in enumerate(J2S):
                    t0 = max(0, -j2)
                    t1 = M - max(0, j2)
                    mm = nc.tensor.matmul(
                        pst[:, :, t0:t1],
                        wtt[c][:, j, :],
                        xt[c][:, b0:b0 + 16, t0 + j2:t1 + j2],
                        start=(j == 0),
                        stop=(j == len(J2S) - 1),
                    )
                mm.then_inc(sm)

        # --- Vector engine: PSUM -> SBUF with bias add.
        nc.vector.wait_ge(sb_, 16)
        for c in range(CL):
            for ph, b0 in ((0, 0), (1, 16)):
                nc.vector.wait_ge(sm, 2 * c + ph + 1)
                nc.vector.tensor_scalar_add(
                    sbo[c][:, b0:b0 + 16, :], ps[(2 * c + ph) % 8][:],
                    bias_t[:, c:c + 1],
                ).then_inc(sc)

        # --- Output DMAs (sync queue, after all x-issues).
        for c in range(CL):
            for ph, b0 in ((0, 0), (1, 16)):
                nc.sync.wait_ge(sc, 2 * c + ph + 1)
                nc.sync.dma_start(
                    out[:, c, b0:b0 + 16], sbo[c][:, b0:b0 + 16]
                ).then_inc(so, 16)

        # --- Teardown: quiesce, reset the data semaphores for re-runs.
        nc.gpsimd.wait_ge(so, 16 * 2 * CL)
        nc.all_engine_barrier()
        sem_nums = sorted(s.num for s in (sx, sw, sb_, sm, sc, so))
        assert sem_nums == list(range(sem_nums[0], sem_nums[0] + 6)), sem_nums
        srange = range(sem_nums[0], sem_nums[-1] + 1)
        nc.gpsimd.dma_reset(srange)
        nc.gpsimd.sem_clear(srange)
        nc.all_engine_barrier()
    nc.finalize()
    return nc


_NC_CACHE = None
LAST_RESULTS = None   # test harness introspection


def kernel(x, lk_w, w0, w1, w2, w3, w4, w5, bn_gamma, bn_beta, bn_mean,
           bn_var):
    global _NC_CACHE, LAST_RESULTS
    x = np.asarray(x, np.float32)
    K, bias = _merge_weights(
        np.asarray(lk_w), [np.asarray(w) for w in (w0, w1, w2, w3, w4, w5)],
        np.asarray(bn_gamma), np.asarray(bn_beta), np.asarray(bn_mean),
        np.asarray(bn_var))
    xs_l, wt_l, bi_l = _stage_inputs(x, K, bias)
    if _NC_CACHE is None:
        _NC_CACHE = _build_program()
    nc = _NC_CACHE
    in_maps = [
        {"xs": xs_l[i], "wt": wt_l[i], "bi": bi_l[i]} for i in range(NCORES)
    ]
    trace = bool(int(os.environ.get("DRB_TRACE", "0")))
    if not trace:
        # NTFF tracing needs the antenv.axon_hooks shim (test.py installs
        # it); make sure a stray BASS_TRACE in the environment can't turn
        # tracing on in a bare harness run.
        os.environ["BASS_NEVER_TRACE"] = "1"
    else:
        os.environ.pop("BASS_NEVER_TRACE", None)
    res = run_bass_kernel_spmd(nc, in_maps, list(range(NCORES)), trace=trace)
    LAST_RESULTS = res
    return _unstage([res.results[i]["out"] for i in range(NCORES)])


# revision 9
# speedup vs baseline: 1.0192x; 1.0192x over previous
"""Trainium2 kernel for nn_DilatedReparamBlock_21139829031531.

Math: the block is 7 depthwise-conv+BN branches summed. All branches merge
("reparameterize") exactly into ONE depthwise 13x13 conv + per-channel bias:
    K_c = sum_i scale_i[c] * dilate(w_i[c])   (placed centered in 13x13)
    bias_c = sum_i (beta_i - mean_i * scale_i)[c]
The host folds weights/BN (tiny), the device runs the single conv.

Device scheme ("row-pair Toeplitz"): channels sharded 32/core across 8 cores.
Per channel, SBUF holds x with partitions = (h%2)*56 + w (a row-PAIR of the
image across 112 partitions). The 13x13 depthwise conv becomes 7 PE matmuls
accumulating in PSUM:
    out[(a_out,w_out), (b,t)] += sum_{a_in,w_in}
        lhsT_j[(a_in,w_in), (a_out,w_out)] * x[b, 2(t+j2)+a_in, w_in]
where lhsT_j[(ai,wi),(ao,wo)] = K[2*j2+ai-ao+6, wi-wo+6] (0 outside the
13-tap band; W/H zero-padding falls out of band clipping + t clipping).
fp16 operands, fp32 PSUM accumulate, fp16 output + host fp32 cast.

Scheduling is hand-rolled raw Bass (no TileContext): every buffer is
statically allocated (no reuse except PSUM's 8-bank rotation), and six
monotonic counting semaphores order the pipeline:
    SX / SW: per-channel x / weight DMA completions (in-order per queue)
    SB:      bias DMA completion
    SM:      tensor marks each (channel, batch-half) PSUM group done
    SC:      vector marks each PSUM->SBUF bias-add copy done
    SO:      output DMA completions (final quiesce waits SO == 64)
This keeps the framework's ~7us entry barrier chain and ~10us teardown
semaphore walk out of the NEFF: the head is just iram load + first DMA,
the tail a single barrier + 6 sem clears + barrier.
"""

import os
import sys

import numpy as np

for _p in ("/opt/trn_rl_repo", "/root/.axon_site/_ro/trn_rl_repo"):
    if os.path.isdir(_p) and _p not in sys.path:
        sys.path.append(_p)

import contextlib

import concourse.bass as bass
import concourse.bacc as bacc
import concourse.mybir as mybir
from concourse.bass_utils import run_bass_kernel_spmd

# ---------------------------------------------------------------- constants
C = 256
B = 32
H = W = 56
M = H // 2            # row pairs
NCORES = 8
CL = C // NCORES      # channels per core
KS = [5, 7, 7, 3, 3, 3]
DIL = [1, 1, 2, 3, 4, 5]
EPS = 1e-5
J2S = [0, -3, -2, -1, 1, 2, 3]   # j2 = (row-pair offset); row offset = 2*j2
F16 = mybir.dt.float16
F32 = mybir.dt.float32
NWARM = 4             # PE clock-ramp warmup matmuls (garbage data)


# ------------------------------------------------------------- host math
def _merge_weights(lk_w, ws, bn_gamma, bn_beta, bn_mean, bn_var):
    """Fold all branches + BN into one [C,13,13] kernel and [C] bias."""
    g = bn_gamma.astype(np.float64)
    be = bn_beta.astype(np.float64)
    mu = bn_mean.astype(np.float64)
    va = bn_var.astype(np.float64)
    scale = g / np.sqrt(va + EPS)          # [7, C]
    shift = be - mu * scale                # [7, C]
    K = np.zeros((C, 13, 13), np.float64)
    K += scale[0][:, None, None] * lk_w[:, 0].astype(np.float64)
    for i, (k, r) in enumerate(zip(KS, DIL)):
        w = ws[i][:, 0].astype(np.float64)
        span = r * (k - 1) + 1
        off = (13 - span) // 2
        ii = off + r * np.arange(k)
        K[:, ii[:, None], ii[None, :]] += scale[i + 1][:, None, None] * w
    bias = shift.sum(axis=0)               # [C]
    return K, bias


def _build_toeplitz(K):
    """lhsT for all channels: [C, 7, 112, 112] fp16.

    lhsT[c, j, p=(ai*56+wi), f=(ao*56+wo)] = K[c, 2*j2+ai-ao+6, wi-wo+6]
    """
    p = np.arange(112)
    f = np.arange(112)
    ai, wi = p // 56, p % 56
    ao, wo = f // 56, f % 56
    dx = wi[:, None] - wo[None, :]                       # [112,112]
    lhs = np.zeros((C, 7, 112, 112), np.float32)
    for j, j2 in enumerate(J2S):
        dy = 2 * j2 + ai[:, None] - ao[None, :]          # [112,112]
        valid = (np.abs(dy) <= 6) & (np.abs(dx) <= 6)
        dyc = np.clip(dy + 6, 0, 12)
        dxc = np.clip(dx + 6, 0, 12)
        vals = K[:, dyc, dxc] * valid[None]              # [C,112,112]
        lhs[:, j] = vals
    return lhs.astype(np.float16)


def _stage_inputs(x, K, bias):
    """Per-core xs [112, CL, B, M] fp16, wt [112, CL, 7, 112] fp16, and a
    bias plane [112, CL] fp32 (bias replicated across partitions so the
    PSUM->SBUF copy can add it as a per-partition tensor_scalar operand).
    Partition counts stay multiples of 16 — the DMA engine striping
    serializes onto one SDMA engine otherwise."""
    lhs = _build_toeplitz(K)                             # [C, 7, 112, 112]
    xr = (
        x.reshape(B, NCORES, CL, M, 2, W)
        .transpose(4, 5, 1, 2, 0, 3)                     # [a, w, core, cl, b, m]
        .reshape(112, NCORES, CL, B, M)
        .astype(np.float16)
    )
    xs_l, wt_l, bi_l = [], [], []
    for core in range(NCORES):
        xs_l.append(np.ascontiguousarray(xr[:, core]))
        wc = lhs[core * CL:(core + 1) * CL].transpose(2, 0, 1, 3)  # [112,CL,7,112]
        wt_l.append(np.ascontiguousarray(wc))
        bc = np.broadcast_to(
            bias[core * CL:(core + 1) * CL].astype(np.float32)[None, :], (112, CL)
        )
        bi_l.append(np.ascontiguousarray(bc))
    return xs_l, wt_l, bi_l


def _unstage(outs):
    """outs: list of 8 arrays [112, CL, B, M] fp16 -> [B, C, H, W] fp32."""
    O = np.stack(outs).astype(np.float32)                # [8, 112, CL, B, M]
    return np.ascontiguousarray(
        O.reshape(NCORES, 2, W, CL, B, M)
        .transpose(4, 0, 3, 5, 1, 2)                     # [B, core, cl, m, a, w]
        .reshape(B, C, H, W)
    )


# --------------------------------------------------------- device program
def _build_program():
    nc = bacc.Bacc()
    xs = nc.declare_dram_parameter("xs", [112, CL, B, M], F16, isOutput=False)
    wt = nc.declare_dram_parameter("wt", [112, CL, 7, 112], F16, isOutput=False)
    bi = nc.declare_dram_parameter("bi", [112, CL], F32, isOutput=False)
    out = nc.declare_dram_parameter("out", [112, CL, B, M], F16, isOutput=True)

    # A DMA's +16 completion increment arrives as 16 per-engine shares, so a
    # single counting semaphore is unsound: wait(16*(c+1)) can be satisfied
    # with later DMAs' early shares substituting for channel c's laggards.
    # A ring of 8 sems per stream makes substitution require an ~8-transfer
    # cross-engine skew, which the uniform packet striping cannot produce.
    sx = [nc.alloc_semaphore(f"sx{k}") for k in range(8)]
    sw = [nc.alloc_semaphore(f"sw{k}") for k in range(8)]
    sb_ = nc.alloc_semaphore("sb")  # bias DMA completion
    sm = nc.alloc_semaphore("sm")   # (channel, half) matmul groups retired
    sc = nc.alloc_semaphore("sc")   # (channel, half) copies retired
    so = nc.alloc_semaphore("so")   # output DMA completions

    ctx = contextlib.ExitStack()
    with ctx:
        xt = [ctx.enter_context(nc.sbuf_tensor(f"xt{c}", [112, B, M], F16))
              for c in range(CL)]
        wtt = [ctx.enter_context(nc.sbuf_tensor(f"wt{c}", [112, 7, 112], F16))
               for c in range(CL)]
        sbo = [ctx.enter_context(nc.sbuf_tensor(f"ot{c}", [112, B, M], F16))
               for c in range(CL)]
        bias_t = ctx.enter_context(nc.sbuf_tensor("bias", [112, CL], F32))
        warm = ctx.enter_context(nc.sbuf_tensor("warm", [112, 512], F16))
        ps = [ctx.enter_context(nc.psum_tensor(f"ps{k}", [112, 16, M], F32))
              for k in range(8)]

        # --- Run-boundary semaphore reset.  The NEFF executes more than once
        # per call (profiling reruns); data sems must start at 0 each run.
        # Warmup matmuls touch no semaphores, so they run before the barrier.
        for _ in range(NWARM):
            nc.tensor.matmul(ps[0][:], warm[:, 0:112], warm[:, 0:448],
                             start=True, stop=True)
        all_sems = sx + sw + [sb_, sm, sc, so]
        sem_nums = sorted(s.num for s in all_sems)
        assert sem_nums == list(
            range(sem_nums[0], sem_nums[0] + len(all_sems))), sem_nums
        nc.gpsimd.sem_clear(range(sem_nums[0], sem_nums[-1] + 1))
        nc.all_engine_barrier()

        # --- DMA issue queues.  sync: x in + results out; gpsimd: weights+bias.
        # Channel 0's transfers get the rings to themselves (the later issues
        # are completion-gated behind them) so the PE's first real matmul
        # isn't delayed by bandwidth-sharing with the rest of the prefetch.
        nc.sync.dma_start(xt[0][:], xs[:, 0]).then_inc(sx[0], 16)
        nc.sync.wait_ge(sx[0], 16)
        for c in range(1, CL):
            nc.sync.dma_start(xt[c][:], xs[:, c]).then_inc(sx[c % 8], 16)
        nc.gpsimd.dma_start(wtt[0][:], wt[:, 0]).then_inc(sw[0], 16)
        nc.gpsimd.wait_ge(sw[0], 16)
        nc.gpsimd.dma_start(bias_t[:], bi[:]).then_inc(sb_, 16)
        for c in range(1, CL):
            nc.gpsimd.dma_start(wtt[c][:], wt[:, c]).then_inc(sw[c % 8], 16)

        # --- Tensor engine: the real matmul stream.
        for c in range(CL):
            nc.tensor.wait_ge(sx[c % 8], 16 * (c // 8 + 1))
            nc.tensor.wait_ge(sw[c % 8], 16 * (c // 8 + 1))
            if c >= 4:
                # PSUM bank pair (2c)%8,(2c+1)%8 last read by channel c-4's
                # copies; SC counts 2 per channel.
                nc.tensor.wait_ge(sc, 2 * c - 6)
            for ph, b0 in ((0, 0), (1, 16)):
                pst = ps[(2 * c + ph) % 8]
                for j, j2 in enumerate(J2S):
                    t0 = max(0, -j2)
                    t1 = M - max(0, j2)
                    mm = nc.tensor.matmul(
                        pst[:, :, t0:t1],
                        wtt[c][:, j, :],
                        xt[c][:, b0:b0 + 16, t0 + j2:t1 + j2],
                        start=(j == 0),
                        stop=(j == len(J2S) - 1),
                    )
                mm.then_inc(sm)

        # --- Vector engine: PSUM -> SBUF with bias add.
        nc.vector.wait_ge(sb_, 16)
        for c in range(CL):
            for ph, b0 in ((0, 0), (1, 16)):
                nc.vector.wait_ge(sm, 2 * c + ph + 1)
                nc.vector.tensor_scalar_add(
                    sbo[c][:, b0:b0 + 16, :], ps[(2 * c + ph) % 8][:],
                    bias_t[:, c:c + 1],
                ).then_inc(sc)

        # --- Output DMAs (sync queue, after all x-issues).
        for c in range(CL):
            for ph, b0 in ((0, 0), (1, 16)):
                nc.sync.wait_ge(sc, 2 * c + ph + 1)
                nc.sync.dma_start(
                    out[:, c, b0:b0 + 16], sbo[c][:, b0:b0 + 16]
                ).then_inc(so, 16)

        # --- Teardown.  The NEFF wrapper's per-engine epilogue zeroes the hw
        # semaphore file in engine-partitioned ranges; the kernel sem range
        # lands in SYNC's share, so only sync must be held back until the
        # output DMAs complete.  Other engines' epilogues overlap the tail.
        nc.sync.wait_ge(so, 16 * 2 * CL)
    nc.finalize()
    return nc


_NC_CACHE = None
LAST_RESULTS = None   # test harness introspection


def kernel(x, lk_w, w0, w1, w2, w3, w4, w5, bn_gamma, bn_beta, bn_mean,
           bn_var):
    global _NC_CACHE, LAST_RESULTS
    x = np.asarray(x, np.float32)
    K, bias = _merge_weights(
        np.asarray(lk_w), [np.asarray(w) for w in (w0, w1, w2, w3, w4, w5)],
        np.asarray(bn_gamma), np.asarray(bn_beta), np.asarray(bn_mean),
        np.asarray(bn_var))
    xs_l, wt_l, bi_l = _stage_inputs(x, K, bias)
    if _NC_CACHE is None:
        _NC_CACHE = _build_program()
    nc = _NC_CACHE
    in_maps = [
        {"xs": xs_l[i], "wt": wt_l[i], "bi": bi_l[i]} for i in range(NCORES)
    ]
    trace = bool(int(os.environ.get("DRB_TRACE", "0")))
    if not trace:
        # NTFF tracing needs the antenv.axon_hooks shim (test.py installs
        # it); make sure a stray BASS_TRACE in the environment can't turn
        # tracing on in a bare harness run.
        os.environ["BASS_NEVER_TRACE"] = "1"
    else:
        os.environ.pop("BASS_NEVER_TRACE", None)
    res = run_bass_kernel_spmd(nc, in_maps, list(range(NCORES)), trace=trace)
    LAST_RESULTS = res
    return _unstage([res.results[i]["out"] for i in range(NCORES)])


# revision 10
# speedup vs baseline: 1.0278x; 1.0084x over previous
"""Trainium2 kernel for nn_DilatedReparamBlock_21139829031531.

Math: the block is 7 depthwise-conv+BN branches summed. All branches merge
("reparameterize") exactly into ONE depthwise 13x13 conv + per-channel bias:
    K_c = sum_i scale_i[c] * dilate(w_i[c])   (placed centered in 13x13)
    bias_c = sum_i (beta_i - mean_i * scale_i)[c]
The host folds weights/BN (tiny), the device runs the single conv.

Device scheme ("row-pair Toeplitz"): channels sharded 32/core across 8 cores.
Per channel, SBUF holds x with partitions = (h%2)*56 + w (a row-PAIR of the
image across 112 partitions). The 13x13 depthwise conv becomes 7 PE matmuls
accumulating in PSUM:
    out[(a_out,w_out), (b,t)] += sum_{a_in,w_in}
        lhsT_j[(a_in,w_in), (a_out,w_out)] * x[b, 2(t+j2)+a_in, w_in]
where lhsT_j[(ai,wi),(ao,wo)] = K[2*j2+ai-ao+6, wi-wo+6] (0 outside the
13-tap band; W/H zero-padding falls out of band clipping + t clipping).
fp16 operands, fp32 PSUM accumulate, fp16 output + host fp32 cast.

Scheduling is hand-rolled raw Bass (no TileContext): every buffer is
statically allocated (no reuse except PSUM's 8-bank rotation), and six
monotonic counting semaphores order the pipeline:
    SX / SW: per-channel x / weight DMA completions (in-order per queue)
    SB:      bias DMA completion
    SM:      tensor marks each (channel, batch-half) PSUM group done
    SC:      vector marks each PSUM->SBUF bias-add copy done
    SO:      output DMA completions (final quiesce waits SO == 64)
This keeps the framework's ~7us entry barrier chain and ~10us teardown
semaphore walk out of the NEFF: the head is just iram load + first DMA,
the tail a single barrier + 6 sem clears + barrier.
"""

import os
import sys

import numpy as np

for _p in ("/opt/trn_rl_repo", "/root/.axon_site/_ro/trn_rl_repo"):
    if os.path.isdir(_p) and _p not in sys.path:
        sys.path.append(_p)

import contextlib

import concourse.bass as bass
import concourse.bacc as bacc
import concourse.mybir as mybir
from concourse.bass_utils import run_bass_kernel_spmd

# ---------------------------------------------------------------- constants
C = 256
B = 32
H = W = 56
M = H // 2            # row pairs
NCORES = 8
CL = C // NCORES      # channels per core
KS = [5, 7, 7, 3, 3, 3]
DIL = [1, 1, 2, 3, 4, 5]
EPS = 1e-5
J2S = [0, -3, -2, -1, 1, 2, 3]   # j2 = (row-pair offset); row offset = 2*j2
F16 = mybir.dt.float16
F32 = mybir.dt.float32
NWARM = 8             # PE clock-ramp warmup matmuls (garbage data)


# ------------------------------------------------------------- host math
def _merge_weights(lk_w, ws, bn_gamma, bn_beta, bn_mean, bn_var):
    """Fold all branches + BN into one [C,13,13] kernel and [C] bias."""
    g = bn_gamma.astype(np.float64)
    be = bn_beta.astype(np.float64)
    mu = bn_mean.astype(np.float64)
    va = bn_var.astype(np.float64)
    scale = g / np.sqrt(va + EPS)          # [7, C]
    shift = be - mu * scale                # [7, C]
    K = np.zeros((C, 13, 13), np.float64)
    K += scale[0][:, None, None] * lk_w[:, 0].astype(np.float64)
    for i, (k, r) in enumerate(zip(KS, DIL)):
        w = ws[i][:, 0].astype(np.float64)
        span = r * (k - 1) + 1
        off = (13 - span) // 2
        ii = off + r * np.arange(k)
        K[:, ii[:, None], ii[None, :]] += scale[i + 1][:, None, None] * w
    bias = shift.sum(axis=0)               # [C]
    return K, bias


def _build_toeplitz(K):
    """lhsT for all channels: [C, 7, 112, 112] fp16.

    lhsT[c, j, p=(ai*56+wi), f=(ao*56+wo)] = K[c, 2*j2+ai-ao+6, wi-wo+6]
    """
    p = np.arange(112)
    f = np.arange(112)
    ai, wi = p // 56, p % 56
    ao, wo = f // 56, f % 56
    dx = wi[:, None] - wo[None, :]                       # [112,112]
    lhs = np.zeros((C, 7, 112, 112), np.float32)
    for j, j2 in enumerate(J2S):
        dy = 2 * j2 + ai[:, None] - ao[None, :]          # [112,112]
        valid = (np.abs(dy) <= 6) & (np.abs(dx) <= 6)
        dyc = np.clip(dy + 6, 0, 12)
        dxc = np.clip(dx + 6, 0, 12)
        vals = K[:, dyc, dxc] * valid[None]              # [C,112,112]
        lhs[:, j] = vals
    return lhs.astype(np.float16)


def _stage_inputs(x, K, bias):
    """Per-core xs [112, CL, B, M] fp16, wt [112, CL, 7, 112] fp16, and a
    bias plane [112, CL] fp32 (bias replicated across partitions so the
    PSUM->SBUF copy can add it as a per-partition tensor_scalar operand).
    Partition counts stay multiples of 16 — the DMA engine striping
    serializes onto one SDMA engine otherwise."""
    lhs = _build_toeplitz(K)                             # [C, 7, 112, 112]
    xr = (
        x.reshape(B, NCORES, CL, M, 2, W)
        .transpose(4, 5, 1, 2, 0, 3)                     # [a, w, core, cl, b, m]
        .reshape(112, NCORES, CL, B, M)
        .astype(np.float16)
    )
    xs_l, wt_l, bi_l = [], [], []
    for core in range(NCORES):
        xs_l.append(np.ascontiguousarray(xr[:, core]))
        wc = lhs[core * CL:(core + 1) * CL].transpose(2, 0, 1, 3)  # [112,CL,7,112]
        wt_l.append(np.ascontiguousarray(wc))
        bc = np.broadcast_to(
            bias[core * CL:(core + 1) * CL].astype(np.float32)[None, :], (112, CL)
        )
        bi_l.append(np.ascontiguousarray(bc))
    return xs_l, wt_l, bi_l


def _unstage(outs):
    """outs: list of 8 arrays [112, CL, B, M] fp16 -> [B, C, H, W] fp32."""
    O = np.stack(outs).astype(np.float32)                # [8, 112, CL, B, M]
    return np.ascontiguousarray(
        O.reshape(NCORES, 2, W, CL, B, M)
        .transpose(4, 0, 3, 5, 1, 2)                     # [B, core, cl, m, a, w]
        .reshape(B, C, H, W)
    )


# --------------------------------------------------------- device program
def _build_program():
    nc = bacc.Bacc()
    xs = nc.declare_dram_parameter("xs", [112, CL, B, M], F16, isOutput=False)
    wt = nc.declare_dram_parameter("wt", [112, CL, 7, 112], F16, isOutput=False)
    bi = nc.declare_dram_parameter("bi", [112, CL], F32, isOutput=False)
    out = nc.declare_dram_parameter("out", [112, CL, B, M], F16, isOutput=True)

    # A DMA's +16 completion increment arrives as 16 per-engine shares, so a
    # single counting semaphore is unsound: wait(16*(c+1)) can be satisfied
    # with later DMAs' early shares substituting for channel c's laggards.
    # A ring of 8 sems per stream makes substitution require an ~8-transfer
    # cross-engine skew, which the uniform packet striping cannot produce.
    sx = [nc.alloc_semaphore(f"sx{k}") for k in range(8)]
    sw = [nc.alloc_semaphore(f"sw{k}") for k in range(8)]
    sb_ = nc.alloc_semaphore("sb")  # bias DMA completion
    sm = nc.alloc_semaphore("sm")   # (channel, half) matmul groups retired
    sc = nc.alloc_semaphore("sc")   # (channel, half) copies retired
    so = nc.alloc_semaphore("so")   # output DMA completions

    ctx = contextlib.ExitStack()
    with ctx:
        xt = [ctx.enter_context(nc.sbuf_tensor(f"xt{c}", [112, B, M], F16))
              for c in range(CL)]
        wtt = [ctx.enter_context(nc.sbuf_tensor(f"wt{c}", [112, 7, 112], F16))
               for c in range(CL)]
        sbo = [ctx.enter_context(nc.sbuf_tensor(f"ot{c}", [112, B, M], F16))
               for c in range(CL)]
        bias_t = ctx.enter_context(nc.sbuf_tensor("bias", [112, CL], F32))
        warm = ctx.enter_context(nc.sbuf_tensor("warm", [112, 512], F16))
        ps = [ctx.enter_context(nc.psum_tensor(f"ps{k}", [112, 16, M], F32))
              for k in range(8)]

        # --- No run-boundary semaphore reset needed: the NEFF wrapper's
        # per-engine epilogue zeroes the entire hw semaphore file after every
        # execution, behind a global rendezvous that our SO-wait holds until
        # the output DMAs complete.  Warmup matmuls ramp the PE clock while
        # the first channel's data is still in flight.
        for _ in range(NWARM):
            nc.tensor.matmul(ps[0][:], warm[:, 0:112], warm[:, 0:448],
                             start=True, stop=True)

        # --- DMA issue queues.  sync: x in + results out; gpsimd: weights+bias.
        # Channel 0's transfers get the rings to themselves (the later issues
        # are completion-gated behind them) so the PE's first real matmul
        # isn't delayed by bandwidth-sharing with the rest of the prefetch.
        nc.sync.dma_start(xt[0][:], xs[:, 0]).then_inc(sx[0], 16)
        nc.sync.wait_ge(sx[0], 16)
        for c in range(1, CL):
            nc.sync.dma_start(xt[c][:], xs[:, c]).then_inc(sx[c % 8], 16)
        nc.gpsimd.dma_start(wtt[0][:], wt[:, 0]).then_inc(sw[0], 16)
        nc.gpsimd.wait_ge(sw[0], 16)
        nc.gpsimd.dma_start(bias_t[:], bi[:]).then_inc(sb_, 16)
        for c in range(1, CL):
            nc.gpsimd.dma_start(wtt[c][:], wt[:, c]).then_inc(sw[c % 8], 16)

        # --- Tensor engine: the real matmul stream.
        for c in range(CL):
            nc.tensor.wait_ge(sx[c % 8], 16 * (c // 8 + 1))
            nc.tensor.wait_ge(sw[c % 8], 16 * (c // 8 + 1))
            if c >= 4:
                # PSUM bank pair (2c)%8,(2c+1)%8 last read by channel c-4's
                # copies; SC counts 2 per channel.
                nc.tensor.wait_ge(sc, 2 * c - 6)
            for ph, b0 in ((0, 0), (1, 16)):
                pst = ps[(2 * c + ph) % 8]
                for j, j2 in enumerate(J2S):
                    t0 = max(0, -j2)
                    t1 = M - max(0, j2)
                    mm = nc.tensor.matmul(
                        pst[:, :, t0:t1],
                        wtt[c][:, j, :],
                        xt[c][:, b0:b0 + 16, t0 + j2:t1 + j2],
                        start=(j == 0),
                        stop=(j == len(J2S) - 1),
                    )
                mm.then_inc(sm)

        # --- Vector engine: PSUM -> SBUF with bias add.
        nc.vector.wait_ge(sb_, 16)
        for c in range(CL):
            for ph, b0 in ((0, 0), (1, 16)):
                nc.vector.wait_ge(sm, 2 * c + ph + 1)
                nc.vector.tensor_scalar_add(
                    sbo[c][:, b0:b0 + 16, :], ps[(2 * c + ph) % 8][:],
                    bias_t[:, c:c + 1],
                ).then_inc(sc)

        # --- Output DMAs (sync queue, after all x-issues).
        for c in range(CL):
            for ph, b0 in ((0, 0), (1, 16)):
                nc.sync.wait_ge(sc, 2 * c + ph + 1)
                nc.sync.dma_start(
                    out[:, c, b0:b0 + 16], sbo[c][:, b0:b0 + 16]
                ).then_inc(so, 16)

        # --- Teardown.  The NEFF wrapper's per-engine epilogue zeroes the hw
        # semaphore file in engine-partitioned ranges; the kernel sem range
        # lands in SYNC's share, so only sync must be held back until the
        # output DMAs complete.  Other engines' epilogues overlap the tail.
        nc.sync.wait_ge(so, 16 * 2 * CL)
    nc.finalize()
    return nc


_NC_CACHE = None
LAST_RESULTS = None   # test harness introspection


def kernel(x, lk_w, w0, w1, w2, w3, w4, w5, bn_gamma, bn_beta, bn_mean,
           bn_var):
    global _NC_CACHE, LAST_RESULTS
    x = np.asarray(x, np.float32)
    K, bias = _merge_weights(
        np.asarray(lk_w), [np.asarray(w) for w in (w0, w1, w2, w3, w4, w5)],
        np.asarray(bn_gamma), np.asarray(bn_beta), np.asarray(bn_mean),
        np.asarray(bn_var))
    xs_l, wt_l, bi_l = _stage_inputs(x, K, bias)
    if _NC_CACHE is None:
        _NC_CACHE = _build_program()
    nc = _NC_CACHE
    in_maps = [
        {"xs": xs_l[i], "wt": wt_l[i], "bi": bi_l[i]} for i in range(NCORES)
    ]
    trace = bool(int(os.environ.get("DRB_TRACE", "0")))
    if not trace:
        # NTFF tracing needs the antenv.axon_hooks shim (test.py installs
        # it); make sure a stray BASS_TRACE in the environment can't turn
        # tracing on in a bare harness run.
        os.environ["BASS_NEVER_TRACE"] = "1"
    else:
        os.environ.pop("BASS_NEVER_TRACE", None)
    res = run_bass_kernel_spmd(nc, in_maps, list(range(NCORES)), trace=trace)
    LAST_RESULTS = res
    return _unstage([res.results[i]["out"] for i in range(NCORES)])
